# revision 1
# baseline (speedup 1.0000x reference)
"""Self-contained Trainium2 kernel for nn_AsyncNaive (ragged multimodal LSTM).

Strategy: the masked LSTM substep is the identity wherever mask[n,t,k]=0, and
sequences are dead (h=c=0, output = const) for t >= seq_length[n]. So each
(modality, sequence) chain is PACKED to its real substeps only (~2*E[seq_len]
~ 257 avg, <= ~550 max, instead of T*K = 1024). The two fusion linears have no
nonlinearity between them, so the head collapses to out = hs @ w + const with
w = W2 @ W1; w is embedded as one extra moving column in the recurrence matmul
(the y column), sampled every substep. The input projection x @ Wih.T (+biases
via a constant-1 row) is fused into the same matmul as extra contraction rows,
filling otherwise-idle PE time. All packed x data is preloaded into SBUF, so
the steady state runs with zero DMA.

Per-core layout (SPMD, linguistic-shaped program on all 6 active cores):
batch = 32 lanes, hidden padded to 300 = 4 quarters of 75. PSUM z tile
[128 = 4 quarters x 32 lanes, 301 = i|f|o|g|y] via 4-way column-tiled matmuls
(tile_position). Cell math in batch-major fp32; h recast bf16 and transposed
for the next substep's stationary via PE transpose + ScalarE copy. Matmul
operands bf16, PSUM accumulation fp32.

Cores: 0-1 linguistic lanes 0-31/32-63, 2-3 acoustic, 4-5 image, 6-7 spares.
"""

import os
import time

import numpy as np

# ---- problem constants (hardcoded; kernel.py must be self-contained) ----
N, T, K = 64, 256, 4
MODS = ["linguistic", "acoustic", "image"]
DIMS = {"linguistic": 300, "acoustic": 88, "image": 128}
HID = {"linguistic": 300, "acoustic": 64, "image": 128}
NCORES = 8
B = 32            # lanes per core
HQ = 75           # hidden quarter (hidden padded to 300 for every modality)
HP = 4 * HQ       # 300
DP = 300          # padded x dim
NC = 4 * HQ + 1   # cols per col-tile: i,f,o,g (75 each) + y
XW = 96           # x cols per substep: 3 blocks of 32 (x-rows 0:128,128:256,256:301)
XROWS = [128, 128, 45]  # x-chunk rows; last = 44 x rows + const-1 bias row


def _build(pmax):
    import concourse.bass as bass
    import concourse.mybir as mybir
    import concourse.tile as tile

    f32 = mybir.dt.float32
    bf16 = mybir.dt.bfloat16
    AF = mybir.ActivationFunctionType
    OP = mybir.AluOpType
    nc = bass.Bass()

    # DRAM: x packed partition-major so the SBUF preload APs line up
    xd = nc.declare_dram_parameter("xpack", [128, pmax, XW], bf16, isOutput=False)
    wrows = [HQ, HQ, HQ, HQ] + XROWS
    wd = [
        nc.declare_dram_parameter(f"w{c}", [wrows[c], 4 * NC], bf16, isOutput=False)
        for c in range(7)
    ]
    idd = nc.declare_dram_parameter("ident", [128, 128], bf16, isOutput=False)
    yout = nc.declare_dram_parameter("yout", [128, pmax], f32, isOutput=True)

    XCH = 32  # substeps per x-preload DMA chunk
    nxch = (pmax + XCH - 1) // XCH

    with tile.TileContext(nc) as tc:
        with (
            tc.tile_pool(name="w", bufs=1) as wpool,
            tc.tile_pool(name="st", bufs=1) as stpool,
            tc.tile_pool(name="g", bufs=3) as gpool,
            tc.tile_pool(name="z", bufs=5, space=bass.MemorySpace.PSUM) as zpool,
            tc.tile_pool(name="tr", bufs=2, space=bass.MemorySpace.PSUM) as trpool,
        ):
            wt = []
            for c in range(7):
                t = wpool.tile([wrows[c], 4 * NC], bf16, tag=f"w{c}")
                nc.sync.dma_start(t[:], wd[c][:, :])
                wt.append(t)
            ident = wpool.tile([128, 128], bf16, tag="ident")
            nc.sync.dma_start(ident[:], idd[:, :])

            # full packed-x stream resident in SBUF, loaded in chunks
            xbig = stpool.tile([128, pmax * XW], bf16, tag="xbig")
            for c in range(nxch):
                p0 = c * XCH
                p1 = min(pmax, p0 + XCH)
                nc.sync.dma_start(
                    xbig[:, p0 * XW:p1 * XW], xd[:, p0:p1, :])

            hT = stpool.tile([HQ, 128], bf16, tag="hT")
            c_st = stpool.tile([128, HQ], f32, tag="c")
            ybuf = stpool.tile([128, pmax], f32, tag="y")
            nc.vector.memset(c_st[:], 0.0)
            nc.vector.memset(hT[:], 0.0)

            def emit_xmms(p, zf):
                xt = xbig[:, p * XW:(p + 1) * XW]
                for g in range(4):
                    zg = zf[32 * g:32 * g + 32, 0:NC]
                    for xc in range(3):
                        rows = XROWS[xc]
                        nc.tensor.matmul(
                            zg, xt[0:rows, 32 * xc:32 * xc + 32],
                            wt[4 + xc][0:rows, g * NC:(g + 1) * NC],
                            start=(xc == 0), stop=False,
                            tile_position=(0, 32 * g), skip_group_check=True,
                        )

            z0 = zpool.tile([128, 512], f32, tag="z", name="z0")
            z1 = zpool.tile([128, 512], f32, tag="z", name="z1")
            z2 = zpool.tile([128, 512], f32, tag="z", name="z2")
            zq = [z0, z1, z2]
            emit_xmms(0, z0)
            emit_xmms(1, z1)
            emit_xmms(2, z2)
            for p in range(pmax):
                zfull = zq.pop(0)
                # h-matmuls split by gate-column range so ACT can start on
                # i|f as soon as those columns land; g|y then o stream later,
                # hidden under the activation tail
                for c0, c1, last in ((0, 150, False), (225, 301, False),
                                     (150, 225, True)):
                    for g in range(4):
                        zg = zfull[32 * g:32 * g + 32, c0:c1]
                        for hq in range(4):
                            nc.tensor.matmul(
                                zg, hT[0:HQ, 32 * hq:32 * hq + 32],
                                wt[hq][0:HQ, g * NC + c0:g * NC + c1],
                                start=False, stop=(last and hq == 3),
                                tile_position=(0, 32 * g), skip_group_check=True,
                            )
                # queue next substep's x-projection ahead of this substep's
                # tail so PE fills its stall window (in-order engine queue)
                if p + 3 < pmax:
                    znext = zpool.tile([128, 512], f32, tag="z", name=f"z{p+3}")
                    emit_xmms(p + 3, znext)
                    zq.append(znext)

                z = zfull[:, 0:NC]
                gact = gpool.tile([128, HP], f32, tag="gact")
                tig = gpool.tile([128, HQ], f32, tag="tig")
                tfc = gpool.tile([128, HQ], f32, tag="tfc")
                ttc = gpool.tile([128, HQ], f32, tag="ttc")
                nc.scalar.activation(gact[:, 0:2 * HQ], z[:, 0:2 * HQ], AF.Sigmoid)
                nc.scalar.activation(gact[:, 3 * HQ:4 * HQ], z[:, 3 * HQ:4 * HQ], AF.Tanh)
                nc.scalar.activation(gact[:, 2 * HQ:3 * HQ], z[:, 2 * HQ:3 * HQ], AF.Sigmoid)
                nc.vector.tensor_tensor(
                    tig[:], gact[:, 0:HQ], gact[:, 3 * HQ:4 * HQ], OP.mult)
                nc.vector.tensor_tensor(
                    tfc[:], gact[:, HQ:2 * HQ], c_st[:], OP.mult)
                nc.vector.tensor_tensor(c_st[:], tig[:], tfc[:], OP.add)
                nc.scalar.activation(ttc[:], c_st[:], AF.Tanh)
                h_st = gpool.tile([128, HQ], bf16, tag="h1")
                nc.vector.tensor_tensor(
                    h_st[:], gact[:, 2 * HQ:3 * HQ], ttc[:], OP.mult)
                if not os.environ.get("KA_NOTR"):
                    trf = trpool.tile([128, 512], bf16, tag="tr")
                    nc.tensor.transpose(trf[0:HQ, 0:128], h_st[:, 0:HQ], ident[:])
                    nc.vector.tensor_copy(hT[0:HQ, :], trf[0:HQ, 0:128])
                nc.vector.tensor_copy(ybuf[:, p:p + 1], z[:, 4 * HQ:4 * HQ + 1])

            nc.sync.dma_start(yout[:, :], ybuf[:])
    return nc


def _split_waits(nc, maxw=1):
    """walrus in this container rejects >1 sync-wait command per instruction;
    split excess semaphore waits onto injected same-engine carrier drains."""
    import concourse.mybir as mybir

    nsplit = 0
    for fn in nc.m.functions:
        for blk in fn.blocks:
            newinsts = []
            for inst in blk.instructions:
                si = getattr(inst, 'sync_info', None)
                w = list(si.on_wait) if si is not None and si.on_wait else []
                if len(w) > maxw:
                    extra, keep = w[:-maxw], w[-maxw:]
                    for j, cw in enumerate(extra):
                        d = mybir.InstDrain(
                            name=f"{inst.name}_wsp{j}",
                            engine=inst.engine,
                            ins=[], outs=[],
                            sync_info=mybir.SyncInfo(on_wait=[cw], on_update=[]),
                        )
                        nc.register_instruction(d, overwrite=True)
                        newinsts.append(d)
                        nsplit += 1
                    inst.sync_info = mybir.SyncInfo(
                        on_wait=keep, on_update=list(si.on_update or []))
                newinsts.append(inst)
            try:
                blk.instructions = newinsts
            except Exception:
                blk.instructions[:] = newinsts
    return nsplit


def _pack_host(inputs):
    """Pack real substeps per (modality, lane); build per-core device inputs."""
    import ml_dtypes

    bf = ml_dtypes.bfloat16
    seq_len = np.asarray(inputs["seq_length"]).astype(np.int64)
    W1 = np.asarray(inputs["fuse_W1"], np.float32)
    W2 = np.asarray(inputs["fuse_W2"], np.float32)
    wcat = (W2 @ W1)[0]  # [492] in concat order ling(300), ac(64), img(128)
    woff = {"linguistic": 0, "acoustic": 300, "image": 364}
    wslice = {m: wcat[woff[m]:woff[m] + HID[m]] for m in MODS}

    packs = {}   # (m, n) -> (tidx [P], kidx [P])
    pmax = 0
    for m in MODS:
        mask = np.asarray(inputs[f"mask_{m}"]).astype(bool)  # [N,T,K]
        for n in range(N):
            L = int(seq_len[n])
            tt, kk = np.nonzero(mask[n, :L])
            packs[(m, n)] = (tt, kk)
            pmax = max(pmax, len(tt))
    pmax += 1  # one flush substep: y is sampled one step late

    core_m = [("linguistic", 0), ("linguistic", 32), ("acoustic", 0),
              ("acoustic", 32), ("image", 0), ("image", 32)]
    in_maps = []
    for m, n0 in core_m:
        Dm, Hm = DIMS[m], HID[m]
        x = np.asarray(inputs[f"x_{m}"], np.float32)
        Wih = np.asarray(inputs[f"Wih_{m}"], np.float32)
        Whh = np.asarray(inputs[f"Whh_{m}"], np.float32)
        bias = (np.asarray(inputs[f"bih_{m}"], np.float32)
                + np.asarray(inputs[f"bhh_{m}"], np.float32))

        # padded [U-rows x gate-cols]; torch gate row order in W: i, f, g, o
        WhhT = np.zeros((HP, 4 * HP), np.float32)   # rows = hid
        WihT = np.zeros((DP + 1, 4 * HP), np.float32)  # rows = x dims + bias row
        for gi in range(4):
            WhhT[0:Hm, gi * HP:gi * HP + Hm] = Whh[gi * Hm:(gi + 1) * Hm, :].T
            WihT[0:Dm, gi * HP:gi * HP + Hm] = Wih[gi * Hm:(gi + 1) * Hm, :].T
            WihT[DP, gi * HP:gi * HP + Hm] = bias[gi * Hm:(gi + 1) * Hm]

        def colblock(wrows_mat):
            # per col-tile quarter g: [i(75), f(75), o(75), g(75), y] columns
            out = np.zeros((wrows_mat.shape[0], 4 * NC), np.float32)
            for g in range(4):
                hs = slice(HQ * g, HQ * (g + 1))
                base = g * NC
                out[:, base + 0 * HQ:base + 1 * HQ] = wrows_mat[:, 0 * HP:1 * HP][:, hs]
                out[:, base + 1 * HQ:base + 2 * HQ] = wrows_mat[:, 1 * HP:2 * HP][:, hs]
                out[:, base + 2 * HQ:base + 3 * HQ] = wrows_mat[:, 3 * HP:4 * HP][:, hs]
                out[:, base + 3 * HQ:base + 4 * HQ] = wrows_mat[:, 2 * HP:3 * HP][:, hs]
            return out

        wy = np.zeros(HP, np.float32)
        wy[0:Hm] = wslice[m]
        mmap = {"ident": np.eye(128, dtype=bf)}
        for hq in range(4):
            blk = colblock(WhhT[HQ * hq:HQ * (hq + 1), :])
            for g in range(4):
                blk[:, g * NC + 4 * HQ] = wy[HQ * hq:HQ * (hq + 1)]
            mmap[f"w{hq}"] = blk.astype(bf)
        xrow0 = [0, 128, 256]
        for xc in range(3):
            mmap[f"w{4 + xc}"] = colblock(
                WihT[xrow0[xc]:xrow0[xc] + XROWS[xc], :]).astype(bf)

        xpack = np.zeros((128, pmax, XW), np.float32)
        xpack[XROWS[2] - 1, :, 64:96] = 1.0  # bias row = const 1
        for b in range(B):
            n = n0 + b
            tt, kk = packs[(m, n)]
            P = len(tt)
            if P == 0:
                continue
            xv = x[n, tt, kk, :]  # [P, Dm]
            r0 = min(Dm, 128)
            xpack[0:r0, 0:P, b] = xv[:, 0:r0].T
            if Dm > 128:
                r1 = min(Dm, 256) - 128
                xpack[0:r1, 0:P, 32 + b] = xv[:, 128:128 + r1].T
            if Dm > 256:
                xpack[0:Dm - 256, 0:P, 64 + b] = xv[:, 256:Dm].T
        mmap["xpack"] = xpack.astype(bf)
        in_maps.append(mmap)

    in_maps.append(in_maps[0])  # cores 6,7: duplicates, outputs ignored
    in_maps.append(in_maps[1])
    return in_maps, packs, pmax, core_m


def _assemble_host(inputs, youts, packs, core_m):
    """ybuf -> per-(m,n,t) head dot samples -> out with the reshape quirk."""
    seq_len = np.asarray(inputs["seq_length"]).astype(np.int64)
    W1 = np.asarray(inputs["fuse_W1"], np.float32)
    W2 = np.asarray(inputs["fuse_W2"], np.float32)
    b1 = np.asarray(inputs["fuse_b1"], np.float32)
    b2 = np.asarray(inputs["fuse_b2"], np.float32)
    const = float(W2[0] @ b1 + b2[0])

    yfull = np.zeros((len(MODS), N, T), np.float32)
    for ci, (m, n0) in enumerate(core_m):
        mi = MODS.index(m)
        # every col-tile's y column accumulates over all 4 h-chunks, so each
        # 32-partition block already holds the FULL dot product per lane
        yb = youts[ci]  # [128, pmax]
        ysum = yb[0:32]  # [32, pmax]
        for b in range(B):
            n = n0 + b
            tt, kk = packs[(m, n)]
            P = len(tt)
            if P == 0:
                continue
            cnt = np.bincount(tt, minlength=T).cumsum()
            L = int(seq_len[n])
            vals = np.concatenate([[0.0], ysum[b, 1:P + 1]])
            yfull[mi, n, 0:L] = vals[cnt[0:L]]
    ystack = yfull.sum(axis=0)  # [N, T]

    # reference quirk: fused[n, t] = hs_stacked[flat n*T + t], stacked = [t', n']
    flat = np.arange(N * T)
    dots = ystack[flat % N, flat // N].reshape(N, T)
    out = (dots + const)[:, :, None] * np.asarray(inputs["lstm_masks"], np.float32)
    return out.astype(np.float32)


def kernel(**inputs):
    t0 = time.time()
    in_maps, packs, pmax, core_m = _pack_host(inputs)
    print(f"[kernel] host pack: {time.time() - t0:.1f}s pmax={pmax}", flush=True)

    nc = _build(pmax)
    ns = _split_waits(nc)
    print(f"[kernel] build: {time.time() - t0:.1f}s (wait-split carriers: {ns})",
          flush=True)

    if os.environ.get("KERNEL_SIM"):
        from concourse.bass_interp import CoreSim

        youts = []
        simtime = 0
        for ci in range(6):
            sim = CoreSim(nc)
            for k, v in in_maps[ci].items():
                sim.tensor(k)[:] = v
            sim.simulate()
            simtime = max(simtime, sim.time)
            youts.append(np.asarray(sim.tensor("yout"), np.float32))
        print(f"[kernel] sim max core time_ns: {simtime}", flush=True)
        try:
            with open("/tmp/kernel_sim_ns.txt", "w") as f:
                f.write(str(simtime))
        except OSError:
            pass
    else:
        from concourse.bass_utils import run_bass_kernel_spmd

        res = run_bass_kernel_spmd(nc, in_maps, core_ids=list(range(NCORES)))
        youts = [np.asarray(res.results[ci]["yout"], np.float32) for ci in range(6)]
    print(f"[kernel] device done: {time.time() - t0:.1f}s", flush=True)

    out = _assemble_host(inputs, youts, packs, core_m)
    print(f"[kernel] total: {time.time() - t0:.1f}s", flush=True)
    return out



# revision 3
# speedup vs baseline: 2.3811x; 2.3811x over previous
"""Trainium2 kernel v2 for nn_AsyncNaive (ragged multimodal LSTM).

Transposed-z design: z^T[gate-dim as PSUM partitions, 32 lanes as free].
In the CoreSim cost model a matmul costs out-free-size rows, so 32-lane
moving operands make each matmul cost 32 rows; partitions and contraction
are free. Per substep: 12 gate-blocks (4 gates x 3 hidden-chunks) x 3
contraction chunks x {x-part, h-part} = 72 matmuls = 2304 rows (~960 ns)
vs 8428 rows in v1. h is stored [hidden parts, lanes] so it is directly
the next matmul's moving operand - the per-substep PE transpose of v1
disappears. The head dot (y) rides as stationary column 1200 on the
(g-gate, chunk2) block, sampled one substep late as in v1.

Gate order in weight cols: i(0:300) f(300:600) o(600:900) g(900:1200) y(1200).
z tile cols: i 0:96 | f 96:192 | o 192:288 | g 288:384, col(G,mu)=G*96+32mu.
Cell math: ACT tanh(g)->gt[:,0:96]; ACT sig(i,f)->gact[:,0:192]; ACT
sig(o)->gact[:,192:288]; DVE P=gact[:,0:192]*gt[:,0:192] (=[i*g, f*c]);
DVE c=P0+P1 (into gt[:,96:192]); ACT th=tanh(c); DVE h=sig(o)*th (bf16).

Cores 0-1 linguistic lanes 0-31/32-63, 2-3 acoustic, 4-5 image (padded to
linguistic shapes), 6-7 spares.
"""

import os
import time

import numpy as np

N, T, K = 64, 256, 4
MODS = ["linguistic", "acoustic", "image"]
DIMS = {"linguistic": 300, "acoustic": 88, "image": 128}
HID = {"linguistic": 300, "acoustic": 64, "image": 128}
NCORES = 8
B = 32
H = 300                      # padded hidden for every modality
D = 301                      # padded x dims + const-1 bias row
GW = 384                     # gate col stride (300 real + 84 zero pad)
WCOLS = 4 * GW               # 1536; y col hides in g-gate pad at 1504
YCOL = 3 * GW + 256 + 96     # 1504 -> partition 96 of the (g, mu=2) block
                             # (engine reads must start at partition 0/32/64/96)
HROWS = [128, 128, 44]       # hidden contraction chunks
XROWS = [128, 128, 45]       # x contraction chunks (last has bias row)
GOFF = [0, GW, 2 * GW, 3 * GW]  # weight col offset per gate (i,f,o,g)
TB = [0, 1, 3, 2]            # torch gate block (i,f,g,o) for our order


def _build(pmax):
    import concourse.bass as bass
    import concourse.mybir as mybir
    import concourse.tile as tile

    f32 = mybir.dt.float32
    bf16 = mybir.dt.bfloat16
    AF = mybir.ActivationFunctionType
    OP = mybir.AluOpType
    nc = bass.Bass()

    PF = int(os.environ.get("KA2_PF", "2"))
    ngrp = (pmax + 15) // 16
    ycols = 128 * ngrp

    xd = nc.declare_dram_parameter("xpack", [128, pmax, 96], bf16, isOutput=False)
    whd = [nc.declare_dram_parameter(f"wh{k}", [HROWS[k], WCOLS], bf16,
                                     isOutput=False) for k in range(3)]
    wxd = [nc.declare_dram_parameter(f"wx{k}", [XROWS[k], WCOLS], bf16,
                                     isOutput=False) for k in range(3)]
    yout = nc.declare_dram_parameter("yout", [128, ycols], f32, isOutput=True)
    dbg_p = int(os.environ.get("KA2_DBG", "-1"))
    if dbg_p >= 0:
        zdbg = nc.declare_dram_parameter("zdbg", [128, 384], f32, isOutput=True)
        gadbg = nc.declare_dram_parameter("gadbg", [128, 288], f32, isOutput=True)
        gtdbg = nc.declare_dram_parameter("gtdbg", [128, 192], f32, isOutput=True)
        hdbg = nc.declare_dram_parameter("hdbg", [128, 96], f32, isOutput=True)

    XCH = 32
    nxch = (pmax + XCH - 1) // XCH

    with tile.TileContext(nc) as tc:
        with (
            tc.tile_pool(name="w", bufs=1) as wpool,
            tc.tile_pool(name="st", bufs=1) as stpool,
            tc.tile_pool(name="ga", bufs=2) as gapool,
            tc.tile_pool(name="pp", bufs=2) as ppool,
            tc.tile_pool(name="th", bufs=2) as thpool,
            tc.tile_pool(name="hh", bufs=3) as hpool,
            tc.tile_pool(name="yb", bufs=2) as ybpool,
            tc.tile_pool(name="zg", bufs=3, space=bass.MemorySpace.PSUM) as zgpool,
            tc.tile_pool(name="zif", bufs=3, space=bass.MemorySpace.PSUM) as zifpool,
            tc.tile_pool(name="zo", bufs=2, space=bass.MemorySpace.PSUM) as zopool,
        ):
            # warmup: start the PE p-state ramp and load the sigmoid/tanh
            # activation tables while the input DMAs stream in
            wdum = wpool.tile([128, 32], bf16, tag="wdum")
            adum = wpool.tile([128, 2], f32, tag="adum")
            nc.vector.memset(wdum[:], 0.0)
            nc.vector.memset(adum[:], 0.0)
            nc.scalar.activation(adum[:, 1:2], adum[:, 0:1], AF.Tanh)
            nc.scalar.activation(adum[:, 1:2], adum[:, 0:1], AF.Sigmoid)

            # spread initial loads across DGE queues so they overlap:
            # x chunks (needed first) on sync, x-weights on gpsimd,
            # h-weights on vector/scalar
            wh = []
            wx = []
            for k in range(3):
                t = wpool.tile([XROWS[k], WCOLS], bf16, tag=f"wx{k}")
                nc.gpsimd.dma_start(t[:], wxd[k][:, :])
                wx.append(t)
            for k in range(3):
                t = wpool.tile([HROWS[k], WCOLS], bf16, tag=f"wh{k}")
                nc.scalar.dma_start(t[:], whd[k][:, :])
                wh.append(t)

            xbig = stpool.tile([128, pmax * 96], bf16, tag="xbig")
            p0 = 0
            for sz in [4, 8, 16]:
                if p0 >= pmax:
                    break
                p1 = min(pmax, p0 + sz)
                nc.sync.dma_start(xbig[:, p0 * 96:p1 * 96], xd[:, p0:p1, :])
                p0 = p1
            while p0 < pmax:
                p1 = min(pmax, p0 + XCH)
                nc.sync.dma_start(xbig[:, p0 * 96:p1 * 96], xd[:, p0:p1, :])
                p0 = p1

            cdt = bf16 if os.environ.get("KA2_BF", "1") == "1" else f32
            gt = stpool.tile([128, 192], cdt, tag="gt")     # tanh(g) | c
            nc.vector.memset(gt[:], 0.0)

            # z is a per-substep TRIPLE of bank-sized PSUM tiles so the ACT
            # reads depend only on their own gate's 9-18 matmuls, and each
            # tile owns its 2KB zero region (start=True safety):
            #   zg: g-gate cols 0:96 (+ y at part 44 of cols 64:96)
            #   zif: i cols 0:96, f cols 96:192
            #   zo: o cols 0:96
            def blkmap(zt, G, mu):
                zg_, zif_, zo_ = zt
                if G == 3:
                    return zg_, 32 * mu
                if G == 0:
                    return zif_, 32 * mu
                if G == 1:
                    return zif_, 96 + 32 * mu
                return zo_, 32 * mu

            def emit_x(p, zt, with_stop):
                seen = set()
                for G in range(4):
                    for mu in range(3):
                        off = GOFF[G] + 128 * mu
                        z_, zc = blkmap(zt, G, mu)
                        for kx in range(3):
                            first = id(z_) not in seen
                            seen.add(id(z_))
                            nc.tensor.matmul(
                                z_[0:128, zc:zc + 32],
                                wx[kx][0:XROWS[kx], off:off + 128],
                                xbig[0:XROWS[kx], p * 96 + 32 * kx:p * 96 + 32 * kx + 32],
                                start=first, stop=(with_stop and kx == 2),
                                skip_group_check=True)

            def emit_h(zt, h):
                for G in (3, 0, 1, 2):  # g first (tanh), then i,f, o last
                    for mu in range(3):
                        off = GOFF[G] + 128 * mu
                        z_, zc = blkmap(zt, G, mu)
                        for k in range(3):
                            nc.tensor.matmul(
                                z_[0:128, zc:zc + 32],
                                wh[k][0:HROWS[k], off:off + 128],
                                h[0:HROWS[k], 32 * k:32 * k + 32],
                                start=False, stop=(k == 2),
                                skip_group_check=True)

            def new_zt(p):
                return (zgpool.tile([128, 512], f32, tag="zg", name=f"zg{p}"),
                        zifpool.tile([128, 512], f32, tag="zif", name=f"zif{p}"),
                        zopool.tile([128, 512], f32, tag="zo", name=f"zo{p}"))

            # a few dummy matmuls pin pe_busy_start near t=0 so the p-state
            # ramp (3us of wall time) completes before the real matmuls start
            zdum = zgpool.tile([128, 512], f32, tag="zg", name="zdum")
            for i in range(3):
                nc.tensor.matmul(zdum[0:32, 0:32], wdum[0:128, 0:32],
                                 wdum[0:128, 0:32], start=(i == 0),
                                 stop=(i == 2), skip_group_check=True)

            ztiles = {}
            for p in range(min(PF, pmax)):
                zt = new_zt(p)
                emit_x(p, zt, with_stop=(p == 0))
                ztiles[p] = zt

            hprev = None
            for p in range(pmax):
                zg, zif, zo = ztiles.pop(p)
                if p > 0:
                    emit_h((zg, zif, zo), hprev)

                gact = gapool.tile([128, 288], cdt, tag="gact")
                nc.scalar.activation(gt[:, 0:96], zg[:, 0:96], AF.Tanh)
                nc.scalar.activation(gact[:, 0:192], zif[:, 0:192], AF.Sigmoid)
                nc.scalar.activation(gact[:, 192:288], zo[:, 0:96], AF.Sigmoid)
                P = ppool.tile([128, 192], cdt, tag="pp")
                nc.vector.tensor_tensor(P[:], gact[:, 0:192], gt[:], OP.mult)
                nc.vector.tensor_tensor(
                    gt[:, 96:192], P[:, 0:96], P[:, 96:192], OP.add)
                if p == 0:
                    ytile = ybpool.tile([128, 128], f32, tag="yb", name="yb0")
                    nc.gpsimd.memset(ytile[:], 0.0)
                if p % 16 == 0 and p + 16 < pmax:
                    # next group's staging tile, allocated a full group early
                    # and memset on the idle gpsimd engine
                    ytnext = ybpool.tile([128, 128], f32, tag="yb",
                                         name=f"yb{p // 16 + 1}")
                    nc.gpsimd.memset(ytnext[:], 0.0)
                if p + PF < pmax:
                    zn = new_zt(p + PF)
                    emit_x(p + PF, zn, with_stop=False)
                    ztiles[p + PF] = zn
                th = thpool.tile([128, 96], cdt, tag="th")
                nc.scalar.activation(th[:], gt[:, 96:192], AF.Tanh)
                hnew = hpool.tile([128, 96], bf16, tag="hh")
                nc.vector.tensor_tensor(hnew[:], gact[:, 192:288], th[:], OP.mult)
                nc.vector.tensor_copy(
                    ytile[32 * (p % 4):32 * (p % 4) + 1,
                          32 * ((p % 16) // 4):32 * ((p % 16) // 4) + 32],
                    zg[96:97, 64:96])
                if p % 16 == 15 or p == pmax - 1:
                    nc.sync.dma_start(
                        yout[:, 128 * (p // 16):128 * (p // 16) + 128], ytile[:])
                    if p + 1 < pmax:
                        ytile = ytnext
                if dbg_p == p:
                    zst = stpool.tile([128, 384], f32, tag="zst")
                    nc.vector.tensor_copy(zst[:, 0:192], zif[:, 0:192])
                    nc.vector.tensor_copy(zst[:, 192:288], zo[:, 0:96])
                    nc.vector.tensor_copy(zst[:, 288:384], zg[:, 0:96])
                    nc.sync.dma_start(zdbg[:, :], zst[:])
                    gts = stpool.tile([128, 192], f32, tag="gts")
                    nc.vector.tensor_copy(gts[:], gt[:])
                    nc.sync.dma_start(gtdbg[:, :], gts[:])
                    hst = stpool.tile([128, 96], f32, tag="hst")
                    nc.vector.tensor_copy(hst[:], hnew[:])
                    nc.sync.dma_start(hdbg[:, :], hst[:])
                hprev = hnew
    return nc


def _split_waits(nc, maxw=1):
    """walrus in this container rejects >1 sync-wait per instruction; split
    excess semaphore waits onto injected same-engine carrier drains."""
    import concourse.mybir as mybir

    nsplit = 0
    for fn in nc.m.functions:
        for blk in fn.blocks:
            newinsts = []
            for inst in blk.instructions:
                si = getattr(inst, 'sync_info', None)
                w = list(si.on_wait) if si is not None and si.on_wait else []
                if len(w) > maxw:
                    extra, keep = w[:-maxw], w[-maxw:]
                    for j, cw in enumerate(extra):
                        d = mybir.InstDrain(
                            name=f"{inst.name}_wsp{j}",
                            engine=inst.engine,
                            ins=[], outs=[],
                            sync_info=mybir.SyncInfo(on_wait=[cw], on_update=[]),
                        )
                        nc.register_instruction(d, overwrite=True)
                        newinsts.append(d)
                        nsplit += 1
                    inst.sync_info = mybir.SyncInfo(
                        on_wait=keep, on_update=list(si.on_update or []))
                newinsts.append(inst)
            try:
                blk.instructions = newinsts
            except Exception:
                blk.instructions[:] = newinsts
    return nsplit


def _pack_host(inputs):
    """Pack real substeps per (modality, lane); build per-core input maps."""
    import ml_dtypes

    bf = ml_dtypes.bfloat16
    seq_len = np.asarray(inputs["seq_length"]).astype(np.int64)
    W1 = np.asarray(inputs["fuse_W1"], np.float32)
    W2 = np.asarray(inputs["fuse_W2"], np.float32)
    wcat = (W2 @ W1)[0]
    woff = {"linguistic": 0, "acoustic": 300, "image": 364}
    wslc = {m: wcat[woff[m]:woff[m] + HID[m]] for m in MODS}

    packs = {}
    pmax = 0
    for m in MODS:
        mask = np.asarray(inputs[f"mask_{m}"]).astype(bool)
        for n in range(N):
            L = int(seq_len[n])
            tt, kk = np.nonzero(mask[n, :L])
            packs[(m, n)] = (tt, kk)
            pmax = max(pmax, len(tt))
    pmax += 1  # flush substep: y sampled one step late

    core_m = [("linguistic", 0), ("linguistic", 32), ("acoustic", 0),
              ("acoustic", 32), ("image", 0), ("image", 32)]
    in_maps = []
    for m, n0 in core_m:
        Dm, Hm = DIMS[m], HID[m]
        x = np.asarray(inputs[f"x_{m}"], np.float32)
        Wih = np.asarray(inputs[f"Wih_{m}"], np.float32)
        Whh = np.asarray(inputs[f"Whh_{m}"], np.float32)
        bias = (np.asarray(inputs[f"bih_{m}"], np.float32)
                + np.asarray(inputs[f"bhh_{m}"], np.float32))

        # weight col layout: gate G block at GOFF[G], y col at 1200
        whm = np.zeros((H, WCOLS), np.float32)    # rows = hidden j
        wxm = np.zeros((D, WCOLS), np.float32)    # rows = x dim (+bias row 300)
        for G in range(4):
            tb = TB[G]
            whm[0:Hm, GOFF[G]:GOFF[G] + Hm] = Whh[tb * Hm:(tb + 1) * Hm, :].T
            wxm[0:Dm, GOFF[G]:GOFF[G] + Hm] = Wih[tb * Hm:(tb + 1) * Hm, :].T
            wxm[300, GOFF[G]:GOFF[G] + Hm] = bias[tb * Hm:(tb + 1) * Hm]
        whm[0:Hm, YCOL] = wslc[m]

        mmap = {}
        for k in range(3):
            mmap[f"wh{k}"] = whm[128 * k:128 * k + HROWS[k], :].astype(bf)
            mmap[f"wx{k}"] = wxm[128 * k:128 * k + XROWS[k], :].astype(bf)

        xpack = np.zeros((128, pmax, 96), np.float32)
        xpack[44, :, 64:96] = 1.0  # bias row (x-dim 300) = const 1
        for b in range(B):
            n = n0 + b
            tt, kk = packs[(m, n)]
            P = len(tt)
            if P == 0:
                continue
            xv = x[n, tt, kk, :]  # [P, Dm]
            r0 = min(Dm, 128)
            xpack[0:r0, 0:P, b] = xv[:, 0:r0].T
            if Dm > 128:
                r1 = min(Dm, 256) - 128
                xpack[0:r1, 0:P, 32 + b] = xv[:, 128:128 + r1].T
            if Dm > 256:
                xpack[0:Dm - 256, 0:P, 64 + b] = xv[:, 256:Dm].T
        mmap["xpack"] = xpack.astype(bf)
        in_maps.append(mmap)

    in_maps.append(in_maps[0])
    in_maps.append(in_maps[1])
    return in_maps, packs, pmax, core_m


def _assemble_host(inputs, youts, packs, core_m, pmax):
    """yout[p%128, 32*(p//128)+lane] = y_{p-1}[lane]; apply reference quirk."""
    seq_len = np.asarray(inputs["seq_length"]).astype(np.int64)
    W1 = np.asarray(inputs["fuse_W1"], np.float32)
    W2 = np.asarray(inputs["fuse_W2"], np.float32)
    b1 = np.asarray(inputs["fuse_b1"], np.float32)
    b2 = np.asarray(inputs["fuse_b2"], np.float32)
    const = float(W2[0] @ b1 + b2[0])

    pidx = np.arange(pmax)
    yfull = np.zeros((len(MODS), N, T), np.float32)
    for ci, (m, n0) in enumerate(core_m):
        mi = MODS.index(m)
        yb = youts[ci]  # [128, ycols]
        # [pmax, 32]: row p, lane l
        ymat = np.stack(
            [yb[32 * (p % 4), 32 * (p // 4):32 * (p // 4) + 32] for p in pidx])
        for b in range(B):
            n = n0 + b
            tt, kk = packs[(m, n)]
            P = len(tt)
            if P == 0:
                continue
            cnt = np.bincount(tt, minlength=T).cumsum()
            L = int(seq_len[n])
            vals = np.concatenate([[0.0], ymat[1:P + 1, b]])
            yfull[mi, n, 0:L] = vals[cnt[0:L]]
    ystack = yfull.sum(axis=0)

    flat = np.arange(N * T)
    dots = ystack[flat % N, flat // N].reshape(N, T)
    out = (dots + const)[:, :, None] * np.asarray(inputs["lstm_masks"], np.float32)
    return out.astype(np.float32)


def kernel(**inputs):
    t0 = time.time()
    in_maps, packs, pmax, core_m = _pack_host(inputs)
    print(f"[kernel] host pack: {time.time() - t0:.1f}s pmax={pmax}", flush=True)

    nc = _build(pmax)
    ns = _split_waits(nc)
    print(f"[kernel] build: {time.time() - t0:.1f}s (wait-split: {ns})", flush=True)

    if os.environ.get("KERNEL_SIM"):
        from concourse.bass_interp import CoreSim

        youts = []
        simtime = 0
        for ci in range(6):
            sim = CoreSim(nc)
            for k, v in in_maps[ci].items():
                sim.tensor(k)[:] = v
            sim.simulate()
            simtime = max(simtime, sim.time)
            youts.append(np.asarray(sim.tensor("yout"), np.float32))
        print(f"[kernel] sim max core time_ns: {simtime}", flush=True)
    else:
        from concourse.bass_utils import run_bass_kernel_spmd

        res = run_bass_kernel_spmd(nc, in_maps, core_ids=list(range(NCORES)))
        youts = [np.asarray(res.results[ci]["yout"], np.float32)
                 for ci in range(6)]
    print(f"[kernel] device done: {time.time() - t0:.1f}s", flush=True)

    out = _assemble_host(inputs, youts, packs, core_m, pmax)
    print(f"[kernel] total: {time.time() - t0:.1f}s", flush=True)
    return out
